# revision 17
# baseline (speedup 1.0000x reference)
"""Multi-head causal attention (B=1, T=4096, D=768, H=12) on 8 trn2 cores.

Sharding: 16 uniform head-slots (2 per core), 12 real heads + 4 dummy
(zero-weight) slots.  Every core runs the IDENTICAL program (SPMD); cores
differ only in the weight data they receive.  Each core computes, for its
two head-slots, the full causal attention over all 4096 tokens plus that
slot-pair's partial output projection.  The host sums the 8 partial
[768, 4096] bf16 outputs, transposes, and adds the output bias.

v3 layout (per core):
  xT    [768, 4096] bf16   x transposed (host supplies), DMA'd per tok-tile
  QT/KT [128, 4096]  bf16  partitions 0:64 slot A dims, 64:128 slot B
  VT    [128, 4096]  bf16  V pre-transpose, then PE-transposed into
  V2    [128, 32*256] bf16 per key chunk c: [V_A | ones x128 | V_B]
                           -> AV matmul lhsT [V_A|ones64] / [ones64|V_B]
                           gives AV rows plus 64x-replicated denominators
  scores in [128 keys, 256 queries] chunks; 6 chunks per ACT exp group;
  av PSUM [128, 512] = ONE bank (A cols 0:256, B cols 256:512), double
  buffered so query tiles overlap.  Normalize: reciprocal_approx_fast of
  the replicated denom block, DRAM-bounce partition broadcast, division
  fused into the PSUM->SBUF copy, merged (k=128) out-proj, bf16 output.
  The issue order is software-pipelined: scores of group i+1 are issued
  before exp/AV of group i so the PE never sits behind the ACT exp.
"""

import math
import numpy as np
import ml_dtypes
from contextlib import ExitStack

import concourse.bass as bass
import concourse.bacc as bacc
import concourse.mybir as mybir
import concourse.tile as tile
from concourse.bass_utils import run_bass_kernel_spmd

BF16 = mybir.dt.bfloat16
F32 = mybir.dt.float32
AF = mybir.ActivationFunctionType

T = 4096
D_MODEL = 768
HEAD_DIM = 64
N_HEADS = 12
N_CORES = 8
QT = 512                  # query tile width
KC = 128                  # key chunk (psum partition dim)
GRP = 2                   # score chunk-jobs per exp group -> ACT free dim 1024
NPAT = 4                  # straddle mask patterns per query tile (QT//KC)
NQT = T // QT             # 16 query tiles
CCH = D_MODEL // 128      # 6 contraction chunks
TOKT = 512                # token tile for projections
NTOKT = T // TOKT
VST = 256                 # V2 stride per 128-key chunk

_PROGRAM_CACHE = {}


def build_program():
    nc = bacc.Bacc(None)

    xT_d = nc.declare_dram_parameter("xT", [D_MODEL, T], BF16, isOutput=False)
    w_d = nc.declare_dram_parameter("wqkv", [3, D_MODEL, 128], BF16, isOutput=False)
    b_d = nc.declare_dram_parameter("bqkv", [128, 3], F32, isOutput=False)
    wo_d = nc.declare_dram_parameter("wo2", [128, D_MODEL], BF16, isOutput=False)
    mk_d = nc.declare_dram_parameter("masks", [NPAT, 128, QT], BF16, isOutput=False)
    id_d = nc.declare_dram_parameter("ident", [128, 128], BF16, isOutput=False)
    outA_d = nc.declare_dram_parameter("outA", [D_MODEL, T], BF16, isOutput=True)
    outB_d = nc.declare_dram_parameter("outB", [D_MODEL, T], BF16, isOutput=True)
    denA_d = nc.declare_dram_parameter("denA", [1, T], BF16, isOutput=True)
    denB_d = nc.declare_dram_parameter("denB", [1, T], BF16, isOutput=True)

    with tile.TileContext(nc) as tc, ExitStack() as ctx:
        consts = ctx.enter_context(tc.tile_pool(name="consts", bufs=1))
        big = ctx.enter_context(tc.tile_pool(name="big", bufs=1))
        ptp = ctx.enter_context(tc.tile_pool(name="ptp", bufs=4))
        rp = ctx.enter_context(tc.tile_pool(name="rp", bufs=2))
        hvp = ctx.enter_context(tc.tile_pool(name="hvp", bufs=4))
        osb = ctx.enter_context(tc.tile_pool(name="osb", bufs=3))
        # PSUM: score/proj/outproj pool 3 banks x2 bufs, av 1 bank x2 = 8
        sp = ctx.enter_context(tc.tile_pool(name="sp", bufs=3, space="PSUM"))
        avp = ctx.enter_context(tc.tile_pool(name="avp", bufs=1, space="PSUM"))
        dramp = ctx.enter_context(tc.tile_pool(name="dramp", bufs=2, space="DRAM"))

        # ---- constants to SBUF ----
        w_sb = consts.tile([128, 3 * CCH * 128], BF16, tag="w")
        for s in range(3):
            for j in range(CCH):
                nc.sync.dma_start(
                    out=w_sb[:, (s * CCH + j) * 128:(s * CCH + j + 1) * 128],
                    in_=w_d[s, j * 128:(j + 1) * 128, :],
                )
        b_sb = consts.tile([128, 3], F32, tag="b")
        nc.sync.dma_start(out=b_sb[:], in_=b_d[:, :])
        wo_sb = consts.tile([128, D_MODEL], BF16, tag="wo")
        nc.sync.dma_start(out=wo_sb[:], in_=wo_d[:, :])
        mask_sb = consts.tile([128, NPAT * QT], BF16, tag="mask")
        for p in range(NPAT):
            nc.sync.dma_start(out=mask_sb[:, p * QT:(p + 1) * QT], in_=mk_d[p, :, :])
        id_sb = consts.tile([128, 128], BF16, tag="id")
        nc.sync.dma_start(out=id_sb[:], in_=id_d[:, :])

        # ---- x input, chunked per (tok tile, contraction chunk) ----
        xT_sb = []
        for j in range(CCH):
            t = big.tile([128, T], BF16, tag=f"xT{j}")
            xT_sb.append(t)
        for tt in range(NTOKT):
            for j in range(CCH):
                nc.sync.dma_start(
                    out=xT_sb[j][:, tt * TOKT:(tt + 1) * TOKT],
                    in_=xT_d[j * 128:(j + 1) * 128, tt * TOKT:(tt + 1) * TOKT],
                )

        # ---- projection / transpose units, interleaved with attention ----
        QT_sb = big.tile([128, T], BF16, tag="Q")
        KT_sb = big.tile([128, T], BF16, tag="K")
        VT_sb = big.tile([128, T], BF16, tag="VT")
        dests = [QT_sb, KT_sb, VT_sb]
        # V2 per 128-token key chunk, stride 256 cols:
        #   [0:64]=V_A  [64:192]=ones  [192:256]=V_B
        # lhsT A = cols 0:128   -> psum rows 0:64 AV_A, 64:128 denom_A (x64)
        # lhsT B = cols 128:256 -> psum rows 0:64 denom_B (x64), 64:128 AV_B
        V_sb = big.tile([128, (T // 128) * VST], BF16, tag="V")
        v3 = V_sb[:].rearrange("p (t c) -> p t c", c=VST)
        nc.vector.memset(v3[:, :, 64:192], 1.0)

        def proj_unit(s, tt):
            def emit():
                pp = sp.tile([128, TOKT], F32, tag="sc", name="pp")
                for j in range(CCH):
                    base = (s * CCH + j) * 128
                    nc.tensor.matmul(
                        pp[:], w_sb[:, base:base + 128],
                        xT_sb[j][:, tt * TOKT:(tt + 1) * TOKT],
                        start=(j == 0), stop=(j == CCH - 1),
                    )
                nc.vector.tensor_scalar_add(
                    dests[s][:, tt * TOKT:(tt + 1) * TOKT],
                    pp[:], b_sb[:, s:s + 1],
                )
            return emit

        def tps_unit(tt):
            def emit():
                for tt4 in range(tt * TOKT // 128, (tt + 1) * TOKT // 128):
                    tp = sp.tile([128, 128], BF16, tag="sc", name="tp")
                    nc.tensor.transpose(
                        tp[:], VT_sb[:, tt4 * 128:(tt4 + 1) * 128], id_sb[:])
                    src = tp[:]
                    dst = V_sb[:, tt4 * VST:(tt4 + 1) * VST]
                    nc.vector.tensor_copy(
                        bass.AP(tensor=dst.tensor, offset=dst.offset,
                                ap=[dst.ap[0], [192, 2], [1, 64]]),
                        bass.AP(tensor=src.tensor, offset=src.offset,
                                ap=[src.ap[0], [64, 2], [1, 64]]),
                    )
            return emit

        av_tiles = {}
        projq = []
        deferred = []

        def issue_scores(G):
            qi, grp, first, last = G
            qs = qi * QT
            sc = sp.tile([128, GRP * QT], F32, tag="sc")
            for ji, (kc, h) in enumerate(grp):
                nc.tensor.matmul(
                    sc[:, ji * QT:(ji + 1) * QT],
                    KT_sb[64 * h:64 * h + 64, kc * KC:(kc + 1) * KC],
                    QT_sb[64 * h:64 * h + 64, qs:qs + QT],
                    start=True, stop=True,
                )
            return sc

        def issue_rest(G, sc):
            qi, grp, first, last = G
            last_qi = qi == NQT - 1
            qs = qi * QT
            nsteps = (qi + 1) * QT // KC
            if first:
                av_tiles[qi] = avp.tile([128, 2 * QT], F32, tag="av", name="av")
            av = av_tiles[qi]
            width = len(grp) * QT
            pt = ptp.tile([128, GRP * QT], BF16, tag="pt")
            nc.scalar.activation(
                pt[:, :width], sc[:, :width], AF.Exp,
                scale=1.0 / math.sqrt(HEAD_DIM),
            )
            for ji, (kc, h) in enumerate(grp):
                ptj = pt[:, ji * QT:(ji + 1) * QT]
                if kc >= nsteps - NPAT:  # diagonal straddle
                    pat = kc - (nsteps - NPAT)
                    m = mask_sb[:, pat * QT:(pat + 1) * QT]
                    nc.vector.tensor_mul(ptj, ptj, m)
                nc.tensor.matmul(
                    av[:, h * QT:(h + 1) * QT],
                    V_sb[:, kc * VST + 128 * h:kc * VST + 128 * h + 128],
                    ptj, start=(kc == 0), stop=(kc == nsteps - 1),
                )
            if projq:
                projq.pop(0)()
            elif deferred:
                deferred.pop(0)()
                if (last_qi or len(deferred) > 12) and deferred:
                    deferred.pop(0)()
            if not last:
                return
            # unnormalized per-slot out-projection; host divides by the
            # denominators (flash-attention-style partial combination).
            # The 12 (matmul+cast+dma) units are deferred and spread one per
            # subsequent score group so the DVE casts never block the masks.
            av = av_tiles.pop(qi)
            hvA = hvp.tile([128, QT], BF16, tag="hvA", name="hvA")
            nc.vector.tensor_copy(hvA[:], av[:, 0:QT])
            hvB = hvp.tile([128, QT], BF16, tag="hvB", name="hvB")
            nc.vector.tensor_copy(hvB[:], av[:, QT:2 * QT])
            nc.sync.dma_start(out=denA_d[0:1, qs:qs + QT], in_=hvA[64:65, :])
            nc.sync.dma_start(out=denB_d[0:1, qs:qs + QT], in_=hvB[0:1, :])

            def mk_op(dch, hv, rows, out_d, qs=qs):
                def emit():
                    op = sp.tile([128, QT], F32, tag="sc", name="op")
                    nc.tensor.matmul(
                        op[:], wo_sb[rows, dch * 128:(dch + 1) * 128], hv[rows, :],
                        start=True, stop=True,
                    )
                    ot = osb.tile([128, QT], BF16, tag="ot", name="ot")
                    nc.vector.tensor_copy(ot[:], op[:])
                    nc.sync.dma_start(
                        out=out_d[dch * 128:(dch + 1) * 128, qs:qs + QT], in_=ot[:],
                    )
                return emit
            for dch in range(CCH):
                deferred.append(mk_op(dch, hvA, slice(0, 64), outA_d))
                deferred.append(mk_op(dch, hvB, slice(64, 128), outB_d))

        # prologue: everything needed by query tile 0
        for s_ in range(3):
            proj_unit(s_, 0)()
        tps_unit(0)()

        from collections import deque
        pend = deque()
        for qi in range(NQT):
            while projq:   # units for tok tiles <= qi must be issued by now
                projq.pop(0)()
            if qi + 1 < NTOKT:
                projq.extend([proj_unit(0, qi + 1), proj_unit(1, qi + 1),
                              proj_unit(2, qi + 1), tps_unit(qi + 1)])
            nsteps = (qi + 1) * QT // KC
            jobs = [(kc, h) for kc in range(nsteps) for h in (0, 1)]
            groups = [(qi, jobs[g0:g0 + GRP], g0 == 0, g0 + GRP >= len(jobs))
                      for g0 in range(0, len(jobs), GRP)]
            for G in groups:
                sc = issue_scores(G)
                pend.append((G, sc))
                if len(pend) > 2:
                    issue_rest(*pend.popleft())
        while pend:
            issue_rest(*pend.popleft())
        while projq:
            projq.pop(0)()
        while deferred:
            deferred.pop(0)()
    nc.finalize()
    return nc


def _host_inputs(x, wq, bq, wk, bk, wv, bv, wo):
    """Per-core input maps. Slot A of core c = head c; slot B = head 8+c
    (cores 0-3) or a dummy zero head (cores 4-7)."""
    bf16 = ml_dtypes.bfloat16
    xT = np.ascontiguousarray(x[0].T).astype(bf16)
    masks = np.zeros((NPAT, 128, QT), np.float32)
    dk = np.arange(128)[:, None]
    dq = np.arange(QT)[None, :]
    for p in range(NPAT):
        masks[p] = (dk + 128 * p <= dq)
    masks = masks.astype(bf16)
    ident = np.eye(128, dtype=np.float32).astype(bf16)

    in_maps = []
    for c in range(N_CORES):
        hA = c
        hB = 8 + c if c < 4 else None
        w = np.zeros((3, D_MODEL, 128), np.float32)
        b = np.zeros((128, 3), np.float32)
        wo2 = np.zeros((128, D_MODEL), np.float32)
        for s, (W, B) in enumerate(((wq, bq), (wk, bk), (wv, bv))):
            w[s, :, 0:64] = W[hA]
            b[0:64, s] = B[hA]
            if hB is not None:
                w[s, :, 64:128] = W[hB]
                b[64:128, s] = B[hB]
        wo2[0:64, :] = wo[hA * 64:(hA + 1) * 64, :]
        if hB is not None:
            wo2[64:128, :] = wo[hB * 64:(hB + 1) * 64, :]
        in_maps.append({
            "xT": xT,
            "wqkv": w.astype(bf16),
            "bqkv": b.astype(np.float32),
            "wo2": wo2.astype(bf16),
            "masks": masks,
            "ident": ident,
        })
    return in_maps


def kernel(_trace=False, _tmpdir=None, **inputs):
    x = np.asarray(inputs["x"], np.float32)
    args = (x,
            np.asarray(inputs["wq"], np.float32), np.asarray(inputs["bq"], np.float32),
            np.asarray(inputs["wk"], np.float32), np.asarray(inputs["bk"], np.float32),
            np.asarray(inputs["wv"], np.float32), np.asarray(inputs["bv"], np.float32),
            np.asarray(inputs["wo"], np.float32))
    bo = np.asarray(inputs["bo"], np.float32)

    if "nc" not in _PROGRAM_CACHE:
        _PROGRAM_CACHE["nc"] = build_program()
    nc = _PROGRAM_CACHE["nc"]

    in_maps = _host_inputs(*args)
    res = run_bass_kernel_spmd(
        nc, in_maps, list(range(N_CORES)), trace=_trace, tmpdir=_tmpdir,
    )
    acc = np.zeros((D_MODEL, T), np.float32)
    for c in range(N_CORES):
        r = res.results[c]
        acc += r["outA"].astype(np.float32) / r["denA"].astype(np.float32)
        acc += r["outB"].astype(np.float32) / r["denB"].astype(np.float32)
    out = acc.T + bo[None, :]
    if _trace:
        return out[None].astype(np.float32), res
    return out[None].astype(np.float32)


# revision 18
# speedup vs baseline: 1.0450x; 1.0450x over previous
"""Multi-head causal attention (B=1, T=4096, D=768, H=12) on 8 trn2 cores.

Sharding: every core runs the IDENTICAL program (SPMD) with two head
slots; cores differ only in weight/input data.
  slot A (partitions 0:64):  one whole head (heads 0..7 on cores 0..7)
  slot B (partitions 64:128): HALF of a split head.  Heads 8..11 are each
    split into even/odd key-chunk interleaves; core 2p gets the even
    chunks of head 8+p, core 2p+1 the odd chunks.  The program always
    processes "key chunk j" of a packed 2048-token stream xT_B whose
    chunks the HOST packed as x chunks 2j+delta — so the even/odd choice
    is pure data.  Per-core mask tables supply the two diagonal-straddle
    patterns ({0,2} for even, {1,3} for odd).
Per-core causal work: 144 (whole) + 72 (half) = 216 key chunks = the
ideal 12*144/8 balance; no dummy compute anywhere.

The kernel emits unnormalized per-slot out-projections plus the softmax
denominators (replicated into a 64-row PSUM block by 64 ones-columns in
the AV lhsT); the host performs the flash-attention-style combination
  out = sum_c O_A_c/den_A_c + sum_p (O_B_2p + O_B_2p+1)/(den_B_2p + den_B_2p+1)

Pipeline: scores are issued with lookahead 2 ahead of exp/AV; projection
and V-transpose units and the out-projection (matmul+cast+DMA) units are
deferred and dripped one per score group so the ACT exp stream (the
throughput floor) never starves and PE matmuls almost always have their
semaphore waits pre-satisfied (back-to-back matmuls stream at ~2.4GHz;
stalled ones serialize at ~1.2GHz effective).
"""

import math
import numpy as np
import ml_dtypes
from contextlib import ExitStack
from collections import deque

import concourse.bass as bass
import concourse.bacc as bacc
import concourse.mybir as mybir
import concourse.tile as tile
from concourse.bass_utils import run_bass_kernel_spmd

BF16 = mybir.dt.bfloat16
F32 = mybir.dt.float32
AF = mybir.ActivationFunctionType

T = 4096
TB = 2048                 # packed token stream length for the half slot
D_MODEL = 768
HEAD_DIM = 64
N_HEADS = 12
N_CORES = 8
QT = 512                  # query tile width
KC = 128                  # key chunk (psum partition dim)
GRP = 2                   # score chunk-jobs per exp group -> ACT free dim 1024
NPAT = 4                  # straddle mask patterns per query tile (QT//KC)
NQT = T // QT             # 8 query tiles
CCH = D_MODEL // 128      # 6 contraction chunks
TOKT = 512                # token tile for projections
NTOKT = T // TOKT
NTOKB = TB // TOKT

_PROGRAM_CACHE = {}


def build_program():
    nc = bacc.Bacc(None)

    xT_d = nc.declare_dram_parameter("xT", [D_MODEL, T], BF16, isOutput=False)
    xB_d = nc.declare_dram_parameter("xB", [D_MODEL, TB], BF16, isOutput=False)
    # weights pre-arranged by host into SBUF layout:
    # wqkv[128, k] columns: s-major (q,k,v), then cch chunk, then 128 head dims
    w_d = nc.declare_dram_parameter("wqkv", [128, 3 * CCH * 128], BF16, isOutput=False)
    b_d = nc.declare_dram_parameter("bqkv", [128, 3], F32, isOutput=False)
    wo_d = nc.declare_dram_parameter("wo2", [128, D_MODEL], BF16, isOutput=False)
    # masks: [128, 6*QT]: 4 slot-A straddle patterns then 2 slot-B patterns
    mk_d = nc.declare_dram_parameter("masks", [128, 6 * QT], BF16, isOutput=False)
    id_d = nc.declare_dram_parameter("ident", [128, 128], BF16, isOutput=False)
    outA_d = nc.declare_dram_parameter("outA", [D_MODEL, T], BF16, isOutput=True)
    outB_d = nc.declare_dram_parameter("outB", [D_MODEL, T], BF16, isOutput=True)
    denA_d = nc.declare_dram_parameter("denA", [1, T], BF16, isOutput=True)
    denB_d = nc.declare_dram_parameter("denB", [1, T], BF16, isOutput=True)

    with tile.TileContext(nc) as tc, ExitStack() as ctx:
        consts = ctx.enter_context(tc.tile_pool(name="consts", bufs=1))
        big = ctx.enter_context(tc.tile_pool(name="big", bufs=1))
        ptp = ctx.enter_context(tc.tile_pool(name="ptp", bufs=6))
        hvp = ctx.enter_context(tc.tile_pool(name="hvp", bufs=4))
        osb = ctx.enter_context(tc.tile_pool(name="osb", bufs=3))
        # PSUM: score/proj/outproj pool 2 banks x3 bufs, av 2 banks x1 = 8
        sp = ctx.enter_context(tc.tile_pool(name="sp", bufs=3, space="PSUM"))
        avp = ctx.enter_context(tc.tile_pool(name="avp", bufs=1, space="PSUM"))

        # ---- constants to SBUF (few large DMAs, off the main data queue) ----
        w_sb = consts.tile([128, 3 * CCH * 128], BF16, tag="w")
        nc.gpsimd.dma_start(out=w_sb[:], in_=w_d[:, :])
        b_sb = consts.tile([128, 3], F32, tag="b")
        nc.gpsimd.dma_start(out=b_sb[:], in_=b_d[:, :])
        wo_sb = consts.tile([128, D_MODEL], BF16, tag="wo")
        nc.gpsimd.dma_start(out=wo_sb[:], in_=wo_d[:, :])
        mask_sb = consts.tile([128, 6 * QT], BF16, tag="mask")
        nc.gpsimd.dma_start(out=mask_sb[:], in_=mk_d[:, :])
        id_sb = consts.tile([128, 128], BF16, tag="id")
        nc.gpsimd.dma_start(out=id_sb[:], in_=id_d[:, :])
        maskA = mask_sb[:, 0:NPAT * QT]
        maskB = mask_sb[:, NPAT * QT:6 * QT]

        # ---- x inputs, chunked per (tok tile, contraction chunk) ----
        xT_sb = []
        xB_sb = []
        for j in range(CCH):
            t = big.tile([128, T], BF16, tag=f"xT{j}", name=f"xT{j}")
            xT_sb.append(t)
            t = big.tile([128, TB], BF16, tag=f"xB{j}", name=f"xB{j}")
            xB_sb.append(t)
        for tt in range(NTOKT):
            for j in range(CCH):
                nc.sync.dma_start(
                    out=xT_sb[j][:, tt * TOKT:(tt + 1) * TOKT],
                    in_=xT_d[j * 128:(j + 1) * 128, tt * TOKT:(tt + 1) * TOKT],
                )
            if tt < NTOKB:
                for j in range(CCH):
                    nc.sync.dma_start(
                        out=xB_sb[j][:, tt * TOKT:(tt + 1) * TOKT],
                        in_=xB_d[j * 128:(j + 1) * 128, tt * TOKT:(tt + 1) * TOKT],
                    )

        # ---- projections ----
        # QT_sb: merged A+B queries (same tokens).  KT_sb/VT_sb: partitions
        # 0:64 = slot A over x, partitions 64:128 = slot B over packed xB.
        QT_sb = big.tile([128, T], BF16, tag="Q")
        KT_sb = big.tile([128, T], BF16, tag="K")
        VT_sb = big.tile([128, T], BF16, tag="VT")
        dests = [QT_sb, KT_sb, VT_sb]
        # V2A per key chunk c: [V_A | ones64] -> AV rows 0:64, den x64 rows 64:128
        # V2B per key chunk c: [ones64 | V_B] -> den x64 rows 0:64, AV rows 64:128
        V2A = big.tile([128, (T // 128) * 128], BF16, tag="V2A")
        V2B = big.tile([128, (TB // 128) * 128], BF16, tag="V2B")
        va3 = V2A[:].rearrange("p (t c) -> p t c", c=128)
        nc.vector.memset(va3[:, :, 64:128], 1.0)
        vb3 = V2B[:].rearrange("p (t c) -> p t c", c=128)
        nc.vector.memset(vb3[:, :, 0:64], 1.0)

        def proj_unit(s, tt, half):
            # s=0 (Q): merged A+B (m=128).  s=1,2 (K,V): half=False -> slot A
            # (m=64, psum rows 0:64) from x; half=True -> slot B (m=64, psum
            # rows 64:128) from the packed xB stream.
            def emit():
                pp = sp.tile([128, TOKT], F32, tag="sc", name="pp")
                for j in range(CCH):
                    base = (s * CCH + j) * 128
                    if s == 0:
                        nc.tensor.matmul(
                            pp[:], w_sb[:, base:base + 128],
                            xT_sb[j][:, tt * TOKT:(tt + 1) * TOKT],
                            start=(j == 0), stop=(j == CCH - 1),
                        )
                    elif not half:
                        nc.tensor.matmul(
                            pp[0:64, :], w_sb[:, base:base + 64],
                            xT_sb[j][:, tt * TOKT:(tt + 1) * TOKT],
                            start=(j == 0), stop=(j == CCH - 1),
                            tile_position=(0, 0),
                        )
                    else:
                        nc.tensor.matmul(
                            pp[64:128, :], w_sb[:, base + 64:base + 128],
                            xB_sb[j][:, tt * TOKT:(tt + 1) * TOKT],
                            start=(j == 0), stop=(j == CCH - 1),
                            tile_position=(0, 64),
                        )
                dst = dests[s][:, tt * TOKT:(tt + 1) * TOKT]
                if s == 0:
                    nc.vector.tensor_scalar_add(dst, pp[:], b_sb[:, s:s + 1])
                elif not half:
                    nc.vector.tensor_scalar_add(
                        dst[0:64], pp[0:64, :], b_sb[0:64, s:s + 1])
                else:
                    nc.vector.tensor_scalar_add(
                        dst[64:128], pp[64:128, :], b_sb[64:128, s:s + 1])
            return emit

        def tps_unit(tt):
            def emit():
                for c in range(tt * TOKT // 128, (tt + 1) * TOKT // 128):
                    tp = sp.tile([128, 128], BF16, tag="sc", name="tp")
                    nc.tensor.transpose(
                        tp[:], VT_sb[:, c * 128:(c + 1) * 128], id_sb[:])
                    nc.vector.tensor_copy(
                        V2A[:, c * 128:c * 128 + 64], tp[:, 0:64])
                    if c < TB // 128:
                        nc.vector.tensor_copy(
                            V2B[:, c * 128 + 64:c * 128 + 128], tp[:, 64:128])
            return emit

        def units_for(tt):
            u = [proj_unit(0, tt, False), proj_unit(1, tt, False),
                 proj_unit(2, tt, False)]
            if tt < NTOKB:
                u += [proj_unit(1, tt, True), proj_unit(2, tt, True)]
            u.append(tps_unit(tt))
            return u

        av_tiles = {}
        projq = []
        deferred = []

        def issue_scores(G):
            qi, grp, first, last = G
            qs = qi * QT
            sc = sp.tile([128, GRP * QT], F32, tag="sc")
            for ji, (kc, h) in enumerate(grp):
                nc.tensor.matmul(
                    sc[:, ji * QT:(ji + 1) * QT],
                    KT_sb[64 * h:64 * h + 64, kc * KC:(kc + 1) * KC],
                    QT_sb[64 * h:64 * h + 64, qs:qs + QT],
                    start=True, stop=True,
                )
            return sc

        def issue_rest(G, sc):
            qi, grp, first, last = G
            last_qi = qi == NQT - 1
            qs = qi * QT
            nstepA = (qi + 1) * QT // KC
            nstepB = nstepA // 2
            if first:
                av_tiles[qi] = avp.tile([128, 2 * QT], F32, tag="av", name="av")
            av = av_tiles[qi]
            width = len(grp) * QT
            pt = ptp.tile([128, GRP * QT], BF16, tag="pt")
            nc.scalar.activation(
                pt[:, :width], sc[:, :width], AF.Exp,
                scale=1.0 / math.sqrt(HEAD_DIM),
            )
            for ji, (kc, h) in enumerate(grp):
                ptj = pt[:, ji * QT:(ji + 1) * QT]
                nstep = nstepB if h else nstepA
                npat = NPAT // 2 if h else NPAT
                mtab = maskB if h else maskA
                if kc >= nstep - npat:  # diagonal straddle
                    pat = kc - (nstep - npat)
                    nc.vector.tensor_mul(
                        ptj, ptj, mtab[:, pat * QT:(pat + 1) * QT])
                v2 = V2B if h else V2A
                nc.tensor.matmul(
                    av[:, h * QT:(h + 1) * QT],
                    v2[:, kc * 128:(kc + 1) * 128],
                    ptj, start=(kc == 0), stop=(kc == nstep - 1),
                )
            if projq:
                projq.pop(0)()
            elif deferred:
                deferred.pop(0)()
                if (last_qi or len(deferred) > 12) and deferred:
                    deferred.pop(0)()
            if not last:
                return
            # unnormalized per-slot out-projection; host divides by the
            # denominators (flash-attention-style partial combination).
            # The 12 (matmul+cast+dma) units are deferred and spread one per
            # subsequent score group so the DVE casts never block the masks.
            av = av_tiles.pop(qi)
            hvA = hvp.tile([128, QT], BF16, tag="hvA", name="hvA")
            nc.vector.tensor_copy(hvA[:], av[:, 0:QT])
            hvB = hvp.tile([128, QT], BF16, tag="hvB", name="hvB")
            nc.vector.tensor_copy(hvB[:], av[:, QT:2 * QT])
            nc.sync.dma_start(out=denA_d[0:1, qs:qs + QT], in_=hvA[64:65, :])
            nc.sync.dma_start(out=denB_d[0:1, qs:qs + QT], in_=hvB[0:1, :])

            def mk_op(dch, hv, rows, out_d, qs=qs):
                def emit():
                    op = sp.tile([128, QT], F32, tag="sc", name="op")
                    nc.tensor.matmul(
                        op[:], wo_sb[rows, dch * 128:(dch + 1) * 128], hv[rows, :],
                        start=True, stop=True,
                    )
                    ot = osb.tile([128, QT], BF16, tag="ot", name="ot")
                    nc.vector.tensor_copy(ot[:], op[:])
                    nc.sync.dma_start(
                        out=out_d[dch * 128:(dch + 1) * 128, qs:qs + QT], in_=ot[:],
                    )
                return emit
            for dch in range(CCH):
                deferred.append(mk_op(dch, hvA, slice(0, 64), outA_d))
                deferred.append(mk_op(dch, hvB, slice(64, 128), outB_d))

        # prologue: everything needed by query tile 0
        for u in units_for(0):
            u()

        pend = deque()
        for qi in range(NQT):
            while projq:   # units for tok tiles <= qi must be issued by now
                projq.pop(0)()
            if qi + 1 < NTOKT:
                projq.extend(units_for(qi + 1))
            nstepA = (qi + 1) * QT // KC
            jobs = ([(kc, 0) for kc in range(nstepA)]
                    + [(kc, 1) for kc in range(nstepA // 2)])
            groups = [(qi, jobs[g0:g0 + GRP], g0 == 0, g0 + GRP >= len(jobs))
                      for g0 in range(0, len(jobs), GRP)]
            for G in groups:
                sc = issue_scores(G)
                pend.append((G, sc))
                if len(pend) > 2:
                    issue_rest(*pend.popleft())
        while pend:
            issue_rest(*pend.popleft())
        while projq:
            projq.pop(0)()
        while deferred:
            deferred.pop(0)()
    nc.finalize()
    return nc


def _host_inputs(x, wq, bq, wk, bk, wv, bv, wo):
    """Per-core input maps.  Slot A of core c = whole head c.  Slot B of
    core 2p+delta = the delta-interleave (even/odd key chunks) of head 8+p."""
    bf16 = ml_dtypes.bfloat16
    x0 = np.ascontiguousarray(x[0].T)          # [768, T]
    xT = x0.astype(bf16)
    # packed interleave streams: chunk j of xB[delta] = x chunk 2j+delta
    xc = x0.reshape(D_MODEL, T // 128, 128)
    xB = {d: np.ascontiguousarray(
        xc[:, d::2, :].reshape(D_MODEL, TB)).astype(bf16) for d in (0, 1)}

    dk = np.arange(128)[:, None]
    dq = np.arange(QT)[None, :]
    ident = np.eye(128, dtype=np.float32).astype(bf16)

    in_maps = []
    for c in range(N_CORES):
        hA = c
        p, delta = divmod(c, 2)
        hB = 8 + p
        # weights pre-arranged into the on-chip SBUF layout
        w = np.zeros((128, 3 * CCH * 128), np.float32)
        b = np.zeros((128, 3), np.float32)
        for s, (W, B) in enumerate(((wq, bq), (wk, bk), (wv, bv))):
            for j in range(CCH):
                base = (s * CCH + j) * 128
                w[:, base:base + 64] = W[hA][j * 128:(j + 1) * 128, :]
                w[:, base + 64:base + 128] = W[hB][j * 128:(j + 1) * 128, :]
            b[0:64, s] = B[hA]
            b[64:128, s] = B[hB]
        wo2 = np.zeros((128, D_MODEL), np.float32)
        wo2[0:64, :] = wo[hA * 64:(hA + 1) * 64, :]
        wo2[64:128, :] = wo[hB * 64:(hB + 1) * 64, :]
        # masks: 4 slot-A patterns (0..3), then 2 slot-B patterns
        masks = np.zeros((128, 6 * QT), np.float32)
        for pat in range(4):
            masks[:, pat * QT:(pat + 1) * QT] = (dk + 128 * pat <= dq)
        for i, pat in enumerate((delta, delta + 2)):
            masks[:, (4 + i) * QT:(5 + i) * QT] = (dk + 128 * pat <= dq)
        in_maps.append({
            "xT": xT,
            "xB": xB[delta],
            "wqkv": w.astype(bf16),
            "bqkv": b.astype(np.float32),
            "wo2": wo2.astype(bf16),
            "masks": masks.astype(bf16),
            "ident": ident,
        })
    return in_maps


def kernel(_trace=False, _tmpdir=None, **inputs):
    x = np.asarray(inputs["x"], np.float32)
    args = (x,
            np.asarray(inputs["wq"], np.float32), np.asarray(inputs["bq"], np.float32),
            np.asarray(inputs["wk"], np.float32), np.asarray(inputs["bk"], np.float32),
            np.asarray(inputs["wv"], np.float32), np.asarray(inputs["bv"], np.float32),
            np.asarray(inputs["wo"], np.float32))
    bo = np.asarray(inputs["bo"], np.float32)

    if "nc" not in _PROGRAM_CACHE:
        _PROGRAM_CACHE["nc"] = build_program()
    nc = _PROGRAM_CACHE["nc"]

    in_maps = _host_inputs(*args)
    res = run_bass_kernel_spmd(
        nc, in_maps, list(range(N_CORES)), trace=_trace, tmpdir=_tmpdir,
    )
    acc = np.zeros((D_MODEL, T), np.float32)
    for c in range(N_CORES):
        r = res.results[c]
        acc += r["outA"].astype(np.float32) / r["denA"].astype(np.float32)
    for p in range(4):
        rA, rB = res.results[2 * p], res.results[2 * p + 1]
        num = rA["outB"].astype(np.float32) + rB["outB"].astype(np.float32)
        den = rA["denB"].astype(np.float32) + rB["denB"].astype(np.float32)
        acc += num / den
    out = acc.T + bo[None, :]
    if _trace:
        return out[None].astype(np.float32), res
    return out[None].astype(np.float32)
